# revision 2
# baseline (speedup 1.0000x reference)
"""GCNConv Trainium2 kernel v2:
out = (segsum_dst(xn[src]) @ W) * norm[dst] + bias,  xn = x * norm  (host-staged bf16)

Distribution: dst-nodes sharded across 8 NeuronCores (12500 each). Per core:
- Edges bucketed by (dst block of 128, src chunk of <=32767) on host.
- Device gathers xn rows per bucket via SWDGE dma_gather spread over 4 SWDGE
  queues (the single-queue ring head ~9ns/row was the old bottleneck; 4 queues
  run at ~2.3ns/row).
- Segment-sum on the tensor engine: acc[f,d] += m_tile^T @ S_tile with S a
  host-shipped fp8 0/1 staircase matrix (exact in fp8), streamed over the
  hardware-DGE queues; no on-device S build (the old DVE tensor_scalar
  S-build was the co-bottleneck).
- Per block: aggT -> bf16, proj = aggT^T @ W (bf16), out = proj*ndst + bias.
"""

import numpy as np

N = 100000
C = 128
NC_ = 8
NPC = N // NC_            # 12500 dst nodes per core
BLK = 128
NBLK = (NPC + BLK - 1) // BLK   # 98 blocks (last has 84 rows)
LAST_ROWS = NPC - (NBLK - 1) * BLK  # 84
CHUNK_BOUNDS = [0, 32767, 65534, 98301, 100000]
NCHUNK = len(CHUNK_BOUNDS) - 1
NBUF = 6                  # gather ring depth per chunk
NQ = 4                    # SWDGE queues

_prog_cache = {}


def _build_program(NT, NUMS):
    """NT[b][c]: S/PE tiles for bucket (b,c); NUMS[b][c]: gather idx count
    (max over cores, rounded to 16, <= NT*128). Slots beyond NUMS are never
    gathered; their S rows are zero so the matmul ignores the stale data."""
    import concourse.bacc as bacc
    import concourse.mybir as mybir
    import concourse.tile as tile
    from concourse.library_config import mlp
    from contextlib import ExitStack

    f32 = mybir.dt.float32
    bf16 = mybir.dt.bfloat16
    fp8 = mybir.dt.float8e4
    i16 = mybir.dt.int16

    TBT = [sum(NT[b]) for b in range(NBLK)]     # tiles per block
    SUMT = sum(TBT)                              # total S tiles
    idx_cols = sum(NUMS[b][c] // 16 for b in range(NBLK) for c in range(NCHUNK))

    nc = bacc.Bacc("TRN2", target_bir_lowering=False, debug=False,
                   num_swdge_queues=NQ)
    xn_d = nc.dram_tensor("xn", [N, C], bf16, kind="ExternalInput")
    idx_d = nc.dram_tensor("idx", [128, idx_cols], i16, kind="ExternalInput")
    s_d = nc.dram_tensor("s8", [128, SUMT * 128], fp8, kind="ExternalInput")
    ndst_d = nc.dram_tensor("ndst", [128, NBLK], f32, kind="ExternalInput")
    w_d = nc.dram_tensor("w", [C, C], bf16, kind="ExternalInput")
    biasb_d = nc.dram_tensor("biasb", [128, C], f32, kind="ExternalInput")
    out_d = nc.dram_tensor("out", [NPC, C], f32, kind="ExternalOutput")

    nc.gpsimd.load_library(mlp)
    with tile.TileContext(nc) as tc, ExitStack() as ctx:
        const = ctx.enter_context(tc.tile_pool(name="const", bufs=1))
        idx_sb = const.tile([128, idx_cols], i16)
        nc.sync.dma_start(idx_sb[:], idx_d.ap()[:])
        ndst_sb = const.tile([128, NBLK], f32)
        nc.sync.dma_start(ndst_sb[:], ndst_d.ap()[:])
        w_sb = const.tile([C, C], bf16)
        nc.sync.dma_start(w_sb[:], w_d.ap()[:])
        biasb_sb = const.tile([128, C], f32)
        nc.sync.dma_start(biasb_sb[:], biasb_d.ap()[:])

        # Persistent gather ring buffers, memset once: slots in [NUMS, nt*128)
        # are never DMA-written; they must start (and stay) finite so the
        # S==0 rows cancel them exactly (0 * NaN would poison the matmul).
        mpool = ctx.enter_context(tc.tile_pool(name="msgs", bufs=1))
        ntmax = [max(NT[b][c] for b in range(NBLK)) for c in range(NCHUNK)]
        mbufs = {}
        for c in range(NCHUNK):
            for i in range(NBUF):
                t = mpool.tile([128, ntmax[c] * C], bf16, tag=f"m{c}_{i}",
                               name=f"mb{c}_{i}")
                nc.vector.memset(t[:], 0.0)
                mbufs[(c, i)] = t
        spool = ctx.enter_context(tc.tile_pool(name="stiles", bufs=1))
        apool = ctx.enter_context(tc.tile_pool(name="aggT", bufs=4))
        opool = ctx.enter_context(tc.tile_pool(name="outt", bufs=4))
        accp = ctx.enter_context(tc.tile_pool(name="acc", bufs=4, space="PSUM"))
        projp = ctx.enter_context(tc.tile_pool(name="proj", bufs=3, space="PSUM"))

        idx_col = 0
        s_col = 0
        qload = [0] * NQ  # greedy least-loaded queue assignment by idx count
        for b in range(NBLK):
            # S tiles for this whole block: one contiguous stream
            st = spool.tile([128, TBT[b] * 128], fp8, tag=f"s{b % NBUF}")
            nc.sync.dma_start(st[:], s_d.ap()[:, s_col * 128:(s_col + TBT[b]) * 128])

            mts = []
            for c in range(NCHUNK):
                L = NUMS[b][c]
                nt = NT[b][c]
                m = mbufs[(c, b % NBUF)]
                q = min(range(NQ), key=lambda j: qload[j])
                qload[q] += L
                nc.gpsimd.dma_gather(
                    out_ap=m[:, : ((L + 127) // 128) * C].rearrange(
                        "p (t f) -> p t f", f=C),
                    in_ap=xn_d.ap()[CHUNK_BOUNDS[c]:CHUNK_BOUNDS[c + 1], :],
                    idxs_ap=idx_sb[:, idx_col: idx_col + L // 16],
                    num_idxs=L,
                    num_idxs_reg=L,
                    elem_size=C,
                    single_packet=(L <= 1024),
                    queue_num=q,
                )
                idx_col += L // 16
                mts.append(m)

            acc = accp.tile([128, 128], f32)
            ti = 0
            for c in range(NCHUNK):
                m = mts[c]
                for u in range(NT[b][c]):
                    nc.tensor.matmul(
                        out=acc[:],
                        lhsT=m[:, u * C:(u + 1) * C],
                        rhs=st[:, ti * 128:(ti + 1) * 128],
                        start=(ti == 0),
                        stop=(ti == TBT[b] - 1),
                    )
                    ti += 1
            s_col += TBT[b]

            aggT = apool.tile([128, 128], bf16)
            nc.scalar.copy(aggT[:], acc[:])
            proj = projp.tile([128, 128], f32)
            nc.tensor.matmul(out=proj[:], lhsT=aggT[:], rhs=w_sb[:],
                             start=True, stop=True)
            outt = opool.tile([128, C], f32)
            nc.vector.scalar_tensor_tensor(
                out=outt[:],
                in0=proj[:],
                scalar=ndst_sb[:, b:b + 1],
                in1=biasb_sb[:],
                op0=mybir.AluOpType.mult,
                op1=mybir.AluOpType.add,
            )
            rows = LAST_ROWS if b == NBLK - 1 else 128
            nc.sync.dma_start(out_d.ap()[b * 128: b * 128 + rows, :], outt[:rows, :])
    nc.compile()
    return nc


def _preprocess(x, norm, weight, bias, edge_src, edge_dst):
    import ml_dtypes

    src = np.asarray(edge_src).astype(np.int64, copy=False).ravel()
    dst = np.asarray(edge_dst).astype(np.int64, copy=False).ravel()
    E = src.size
    normf = np.asarray(norm, dtype=np.float32).ravel()

    xn = (np.asarray(x, dtype=np.float32) * normf[:, None]).astype(
        ml_dtypes.bfloat16)

    core = dst // NPC
    rem = dst - core * NPC
    blk = rem >> 7
    dstl = (rem & 127).astype(np.int64)
    bounds = np.asarray(CHUNK_BOUNDS, dtype=np.int64)
    chunk = np.searchsorted(bounds, src, side="right") - 1
    lsrc = (src - bounds[chunk]).astype(np.int16)

    key = ((core * NBLK + blk) * NCHUNK + chunk).astype(np.int64)
    # dedup within bucket: edges sharing (bucket, src) use one gathered slot;
    # S rows then carry one 1 per distinct dst (or small ints for multi-edges,
    # still exact in fp8)
    comb = key * 131072 + src
    order = np.argsort(comb, kind="stable")
    key_o = key[order]
    new_grp = np.ones(E, bool)
    new_grp[1:] = comb[order][1:] != comb[order][:-1]
    gid = np.cumsum(new_grp) - 1
    new_bucket = np.ones(E, bool)
    new_bucket[1:] = key_o[1:] != key_o[:-1]
    bucket_first_gid = np.maximum.accumulate(np.where(new_bucket, gid, -1))
    rank = gid - bucket_first_gid                       # unique-slot rank
    ucnt = np.bincount(key_o[new_grp], minlength=NC_ * NBLK * NCHUNK)
    cnt3 = ucnt.reshape(NC_, NBLK, NCHUNK)

    # Shared (max-over-cores) per-bucket gather counts and tile counts
    mx = cnt3.max(axis=0)                               # [NBLK, NCHUNK]
    NUMS = [[int(-(-max(1, int(mx[b, c])) // 16) * 16) for c in range(NCHUNK)]
            for b in range(NBLK)]
    NT = [[(NUMS[b][c] + 127) // 128 for c in range(NCHUNK)] for b in range(NBLK)]
    TBT = [sum(NT[b]) for b in range(NBLK)]
    SUMT = sum(TBT)

    # gather slot capacity layout: bucket (core,b,c) occupies NUMS[b][c] idx
    # slots; S layout: same bucket occupies NT[b][c]*128 S-rows at tile-aligned
    # positions. rank < NUMS always (NUMS >= max count).
    nums_f = np.array(NUMS, dtype=np.int64).ravel()          # [NBLK*NCHUNK]
    nt_f = np.array(NT, dtype=np.int64).ravel() * 128
    idx_starts = np.concatenate([[0], np.cumsum(nums_f)[:-1]])
    s_starts = np.concatenate([[0], np.cumsum(nt_f)[:-1]])
    # S tile base offset per block (tiles packed by block then chunk)
    # s_starts above is per (b,c) within the global S stream since tiles are
    # packed in (b,c) order with nt_f rows each: consistent.

    bc = (blk * NCHUNK + chunk)[order]                        # [E] bucket id
    core_o = core[order]
    lsrc_o = lsrc[order]
    dstl_o = dstl[order]

    idx_tot = int(nums_f.sum())
    s_rows_tot = int(nt_f.sum())  # = SUMT*128

    # ---- per-core idx tables (wrapped int16) ----
    idx_cols = idx_tot // 16
    idx_all = np.zeros((NC_, idx_tot), np.int16)
    slot_idx = idx_starts[bc] + rank
    idx_all[core_o, slot_idx] = lsrc_o

    idx_w = np.empty((NC_, 128, idx_cols), np.int16)
    for k in range(NC_):
        a16 = idx_all[k].reshape(-1, 16).T                    # [16, idx_tot/16]
        idx_w[k] = np.tile(a16, (8, 1))

    # ---- per-core S (fp8 small-int counts) ----
    # S rows: [128, SUMT*128] where row r (partition r) of tile t has the
    # multi-edge count at column (t*128 + dstl) for the slot (tile t, row r).
    s_all = np.zeros((NC_, s_rows_tot, 128), np.uint8)
    es = s_starts[bc] + rank                                 # global S row
    np.add.at(s_all, (core_o, es, dstl_o), 1)
    # reshape to [NC, SUMT, 128, 128] -> partition-major [NC, 128, SUMT*128]
    s_resh = s_all.reshape(NC_, SUMT, 128, 128).transpose(0, 2, 1, 3).reshape(
        NC_, 128, SUMT * 128)
    s8 = s_resh.astype(np.float32).astype(ml_dtypes.float8_e4m3)

    biasb = np.broadcast_to(np.asarray(bias, np.float32), (128, C)).copy()
    wbf = np.asarray(weight, dtype=np.float32).astype(ml_dtypes.bfloat16)

    nd_full = np.zeros((NC_, NBLK * 128), np.float32)
    nd_full[:, :NPC] = normf.reshape(NC_, NPC)
    ndst = nd_full.reshape(NC_, NBLK, 128).transpose(0, 2, 1).copy()

    in_maps = []
    for k in range(NC_):
        in_maps.append({
            "xn": xn,
            "idx": np.ascontiguousarray(idx_w[k]),
            "s8": np.ascontiguousarray(s8[k]),
            "ndst": np.ascontiguousarray(ndst[k]),
            "w": wbf,
            "biasb": biasb,
        })
    return NT, NUMS, in_maps


def _run(inputs, trace=False, trace_kwargs=None):
    from concourse.bass_utils import run_bass_kernel_spmd

    NT, NUMS, in_maps = _preprocess(**inputs)
    key = tuple(tuple(r) for r in NUMS)
    if key not in _prog_cache:
        _prog_cache[key] = _build_program(NT, NUMS)
    nc = _prog_cache[key]
    kw = {}
    if trace:
        kw["trace"] = True
        if trace_kwargs:
            kw["trace_kwargs"] = trace_kwargs
    res = run_bass_kernel_spmd(nc, in_maps, core_ids=list(range(NC_)), **kw)
    out = np.concatenate([res.results[k]["out"] for k in range(NC_)], axis=0)
    return out, res


def kernel(**inputs):
    out, _ = _run(inputs, trace=False)
    return out
